# revision 9
# baseline (speedup 1.0000x reference)
"""Trainium2 Bass kernel for the AttentionLayer problem.

Math (per batch):
    Q = inp_q @ Wq + bq            [S, d]
    K = inp_k @ Wk + bk            [S, d]
    V = inp_v @ Wv + bv            [S, d]
    sc = Q @ K^T / sqrt(d)         [Sq, Sk]
    S_ = softmax(sc, axis=0)       (over the QUERY axis)
    H = S_ @ V                     [Sq, d]

Device-side layout strategy (per core, 2 batches):
  * Host feeds transposed activations xT = x^T [D, S] so every matmul
    contracts over the SBUF partition dim with zero on-chip transposes
    of the big activations.
  * Projections produce QT/KT/VT in [d, S] layout (d = 128 partitions).
  * scores^T [k, q] = (KT-slice)^T @ QT, so softmax-over-q is a
    free-axis row reduction: one ACT pass does exp(scale*x) and the
    row sum Z[k].  No max-subtraction is needed: |sc/sqrt(d)| <~ 6 for
    randn inputs, exp() is exact in f32 there.
  * Normalization is folded into V: vs[k, :] = V[k, :] / Z[k], then
    H^T [d, q] += vs-slice^T @ P^T accumulates over k-chunks in PSUM.
  * Host un-transposes H^T -> H.
Compute dtype bf16 (f32 PSUM accumulate), stats in f32.
"""

import math
import sys

sys.path.insert(0, "/opt/trn_rl_repo")

import numpy as np

import concourse.bass as bass  # noqa: E402
import concourse.tile as tile  # noqa: E402
from concourse import bacc, mybir  # noqa: E402
from concourse.masks import make_identity  # noqa: E402

P = 128          # partitions / head dim d
S = 2048         # sequence length
D = 1024         # model dim
DC = D // P      # D chunks (8)
KC = S // P      # key chunks (16)
B_LOC = 2        # batches per core
N_CORES = 8
SCALE = 1.0 / math.sqrt(P)

F32 = mybir.dt.float32
BF16 = mybir.dt.bfloat16

_BUILT = None  # cached (nc,) so repeated kernel() calls reuse the NEFF


def build():
    nc = bacc.Bacc("TRN2", target_bir_lowering=False, debug=False,
                   num_devices=N_CORES)

    dr_in = {}
    for t in ("q", "k", "v"):
        dr_in[t] = nc.dram_tensor(f"{t}T", [B_LOC, D, S], F32,
                                  kind="ExternalInput")
    dr_w = {t: nc.dram_tensor(f"w{t}", [D, P], F32, kind="ExternalInput")
            for t in ("q", "k", "v")}
    dr_b = {t: nc.dram_tensor(f"b{t}", [P], F32, kind="ExternalInput")
            for t in ("q", "k", "v")}
    dr_out = nc.dram_tensor("out", [B_LOC, P, S], F32, kind="ExternalOutput")

    with tile.TileContext(nc) as tc:
        with (
            tc.tile_pool(name="const", bufs=1) as const,
            tc.tile_pool(name="stream", bufs=12) as stream,
            tc.tile_pool(name="proj", bufs=2) as proj,
            tc.tile_pool(name="ptp", bufs=14) as ptp,
            tc.tile_pool(name="stats", bufs=4) as stats,
            tc.tile_pool(name="recp", bufs=18) as recp,
            tc.tile_pool(name="osb", bufs=2) as osb,
            tc.tile_pool(name="ps_big", bufs=2, space="PSUM") as ps_big,
            tc.tile_pool(name="ps_acc", bufs=1, space="PSUM") as ps_acc,
        ):
            # ---- constants ----
            w_sb = {}
            b_sb = {}
            for t in ("q", "k", "v"):
                w_sb[t] = const.tile([P, DC, P], BF16, tag=f"w{t}", name=f"w{t}")
                # [D, P] -> [p, c, e]; cast f32->bf16 inline (SWDGE)
                nc.gpsimd.dma_start(
                    w_sb[t][:],
                    dr_w[t].ap().rearrange("(c p) e -> p c e", p=P))
                b_sb[t] = const.tile([P, 1], F32, tag=f"b{t}", name=f"b{t}")
                nc.sync.dma_start(
                    b_sb[t][:],
                    dr_b[t].ap().rearrange("(p o) -> p o", o=1))
            ident = const.tile([P, P], BF16, tag="ident", name="ident")
            make_identity(nc, ident[:])

            for b in range(B_LOC):
                # ---- Q projection (8 full-S chunks) ----
                halves = [ps_big.tile([P, 1024], F32, tag="big", name="q_ps")
                          for _ in range(2)]
                for c in range(DC):
                    x = stream.tile([P, S], BF16, tag="stream", name="x")
                    nc.gpsimd.dma_start(x[:], dr_in["q"].ap()[b, c * P:(c + 1) * P, :])
                    for h in range(2):
                        for s2 in range(2):
                            nc.tensor.matmul(
                                halves[h][:, s2 * 512:(s2 + 1) * 512],
                                lhsT=w_sb["q"][:, c, :],
                                rhs=x[:, h * 1024 + s2 * 512:
                                      h * 1024 + (s2 + 1) * 512],
                                start=(c == 0), stop=(c == DC - 1))
                qt = proj.tile([P, S], BF16, tag="qT", name="qT")
                for h in range(2):
                    nc.vector.tensor_scalar_add(
                        qt[:, h * 1024:(h + 1) * 1024],
                        halves[h][:], b_sb["q"][:])

                # ---- K projection in two S-halves; scores/exp for a
                # half's 8 k-chunks start as soon as that half's KT is
                # done (k half-chunks are 0.5MB DMAs, arriving earlier
                # than a full-K wait would allow). ----
                pts = []
                recs = []
                for hh in range(2):
                    ktile = proj.tile([P, 1024], BF16, tag="kt", name="kt")
                    kps = ps_big.tile([P, 1024], F32, tag="big", name="k_ps")
                    for c in range(DC):
                        xk = stream.tile([P, 1024], BF16, tag="stream",
                                         name="xk")
                        nc.gpsimd.dma_start(
                            xk[:], dr_in["k"].ap()[b, c * P:(c + 1) * P,
                                                   hh * 1024:(hh + 1) * 1024])
                        for s2 in range(2):
                            nc.tensor.matmul(
                                kps[:, s2 * 512:(s2 + 1) * 512],
                                lhsT=w_sb["k"][:, c, :],
                                rhs=xk[:, s2 * 512:(s2 + 1) * 512],
                                start=(c == 0), stop=(c == DC - 1))
                    nc.vector.tensor_scalar_add(ktile[:], kps[:], b_sb["k"][:])

                    for kc in range(hh * 8, (hh + 1) * 8):
                        pt = ptp.tile([P, S], BF16, tag="pt", name="pt")
                        zz = stats.tile([P, 2], F32, tag="z", name="zz")
                        for h in range(2):
                            sc = ps_big.tile([P, 1024], F32, tag="big",
                                             name="sc_ps")
                            for s2 in range(2):
                                nc.tensor.matmul(
                                    sc[:, s2 * 512:(s2 + 1) * 512],
                                    lhsT=ktile[:, (kc % 8) * P:
                                               (kc % 8 + 1) * P],
                                    rhs=qt[:, h * 1024 + s2 * 512:
                                           h * 1024 + (s2 + 1) * 512],
                                    start=True, stop=True)
                            nc.scalar.activation(
                                pt[:, h * 1024:(h + 1) * 1024], sc[:],
                                func=mybir.ActivationFunctionType.Exp,
                                scale=SCALE, accum_out=zz[:, h:h + 1])
                        rec = recp.tile([P, 1], F32, tag="rec", name="rec")
                        nc.vector.tensor_reduce(
                            rec[:], zz[:], axis=mybir.AxisListType.X,
                            op=mybir.AluOpType.add)
                        nc.vector.reciprocal(rec[:], rec[:])
                        pts.append(pt)
                        recs.append(rec)

                # ---- V projection into the "acc" PSUM region (time-shared
                # VT -> transpose -> HT; never contends with "big"). The
                # v-chunk DMAs land while the scores/exp chain runs. ----
                vt_ps = ps_acc.tile([P, S], F32, tag="acc", name="vt_ps")
                for c in range(DC):
                    x = stream.tile([P, S], BF16, tag="stream", name="x")
                    nc.gpsimd.dma_start(
                        x[:], dr_in["v"].ap()[b, c * P:(c + 1) * P, :])
                    for r in range(4):
                        nc.tensor.matmul(
                            vt_ps[:, r * 512:(r + 1) * 512],
                            lhsT=w_sb["v"][:, c, :],
                            rhs=x[:, r * 512:(r + 1) * 512],
                            start=(c == 0), stop=(c == DC - 1))

                vt_sb = proj.tile([P, S], BF16, tag="vT", name="vT")
                for h in range(2):
                    nc.vector.tensor_scalar_add(
                        vt_sb[:, h * 1024:(h + 1) * 1024],
                        vt_ps[:, h * 1024:(h + 1) * 1024], b_sb["v"][:])
                v_sb = proj.tile([P, KC, P], BF16, tag="v", name="v")
                trt = ps_acc.tile([P, S], BF16, tag="acc", name="tr_ps")
                for idx in range(KC):
                    nc.tensor.transpose(
                        trt[:, idx * P:(idx + 1) * P],
                        vt_sb[:, idx * P:(idx + 1) * P], ident[:])
                for g in range(2):
                    nc.vector.tensor_copy(
                        v_sb[:, g * 8:(g + 1) * 8, :],
                        trt[:, g * 1024:(g + 1) * 1024].rearrange(
                            "p (a e) -> p a e", a=8))

                # ---- H^T accumulation over k-chunks ----
                ht = ps_acc.tile([P, S], F32, tag="acc", name="ht")
                for kc in range(KC):
                    vs = stats.tile([P, P], BF16, tag="vs", name="vs")
                    nc.vector.tensor_scalar_mul(
                        vs[:], v_sb[:, kc, :], recs[kc][:])
                    for st in range(4):
                        nc.tensor.matmul(
                            ht[:, st * 512:(st + 1) * 512],
                            lhsT=vs[:],
                            rhs=pts[kc][:, st * 512:(st + 1) * 512],
                            start=(kc == 0), stop=(kc == KC - 1))
                out_sb = osb.tile([P, S], F32, tag="osb", name="out_sb")
                nc.vector.tensor_copy(out_sb[:], ht[:])
                nc.sync.dma_start(dr_out.ap()[b], out_sb[:])

    nc.compile()
    return nc


def _get_nc():
    global _BUILT
    if _BUILT is None:
        _BUILT = build()
    return _BUILT


def kernel(inp_q, inp_k, inp_v, Wq_kernel, Wq_bias, Wk_kernel, Wk_bias,
           Wv_kernel, Wv_bias):
    from concourse.bass_utils import run_bass_kernel_spmd

    nc = _get_nc()

    inp = {"q": np.asarray(inp_q, dtype=np.float32),
           "k": np.asarray(inp_k, dtype=np.float32),
           "v": np.asarray(inp_v, dtype=np.float32)}
    w = {"q": np.ascontiguousarray(np.asarray(Wq_kernel, dtype=np.float32)),
         "k": np.ascontiguousarray(np.asarray(Wk_kernel, dtype=np.float32)),
         "v": np.ascontiguousarray(np.asarray(Wv_kernel, dtype=np.float32))}
    bias = {"q": np.ascontiguousarray(np.asarray(Wq_bias, dtype=np.float32)),
            "k": np.ascontiguousarray(np.asarray(Wk_bias, dtype=np.float32)),
            "v": np.ascontiguousarray(np.asarray(Wv_bias, dtype=np.float32))}

    in_maps = []
    for c in range(N_CORES):
        m = {}
        for t in ("q", "k", "v"):
            # [2, S, D] -> [2, D, S] contiguous (pure layout marshalling)
            m[f"{t}T"] = np.ascontiguousarray(
                inp[t][c * B_LOC:(c + 1) * B_LOC].transpose(0, 2, 1))
            m[f"w{t}"] = w[t]
            m[f"b{t}"] = bias[t]
        in_maps.append(m)

    res = run_bass_kernel_spmd(nc, in_maps, list(range(N_CORES)))

    out = np.empty((N_CORES * B_LOC, S, P), dtype=np.float32)
    for c in range(N_CORES):
        # [2, P, S] -> [2, S, P]
        out[c * B_LOC:(c + 1) * B_LOC] = (
            res.results[c]["out"].transpose(0, 2, 1))
    return out


# revision 10
# speedup vs baseline: 1.1196x; 1.1196x over previous
"""Trainium2 Bass kernel for the AttentionLayer problem.

Math (per batch):
    Q = inp_q @ Wq + bq            [S, d]
    K = inp_k @ Wk + bk            [S, d]
    V = inp_v @ Wv + bv            [S, d]
    sc = Q @ K^T / sqrt(d)         [Sq, Sk]
    S_ = softmax(sc, axis=0)       (over the QUERY axis)
    H = S_ @ V                     [Sq, d]

Device-side layout strategy (per core, 2 batches):
  * Host feeds transposed activations xT = x^T [D, S] so every matmul
    contracts over the SBUF partition dim with zero on-chip transposes
    of the big activations.
  * Projections produce QT/KT/VT in [d, S] layout (d = 128 partitions).
  * scores^T [k, q] = (KT-slice)^T @ QT, so softmax-over-q is a
    free-axis row reduction: one ACT pass does exp(scale*x) and the
    row sum Z[k].  No max-subtraction is needed: |sc/sqrt(d)| <~ 6 for
    randn inputs, exp() is exact in f32 there.
  * Normalization is folded into V: vs[k, :] = V[k, :] / Z[k], then
    H^T [d, q] += vs-slice^T @ P^T accumulates over k-chunks in PSUM.
  * Host un-transposes H^T -> H.
Compute dtype bf16 (f32 PSUM accumulate), stats in f32.
"""

import math
import sys

sys.path.insert(0, "/opt/trn_rl_repo")

import numpy as np

import concourse.bass as bass  # noqa: E402
import concourse.tile as tile  # noqa: E402
from concourse import bacc, mybir  # noqa: E402
from concourse.masks import make_identity  # noqa: E402

P = 128          # partitions / head dim d
S = 2048         # sequence length
D = 1024         # model dim
DC = D // P      # D chunks (8)
KC = S // P      # key chunks (16)
B_LOC = 2        # batches per core
N_CORES = 8
SCALE = 1.0 / math.sqrt(P)

F32 = mybir.dt.float32
BF16 = mybir.dt.bfloat16

_BUILT = None  # cached (nc,) so repeated kernel() calls reuse the NEFF


def build():
    nc = bacc.Bacc("TRN2", target_bir_lowering=False, debug=False,
                   num_devices=N_CORES)

    dr_in = {}
    for t in ("q", "k", "v"):
        dr_in[t] = nc.dram_tensor(f"{t}T", [B_LOC, D, S], F32,
                                  kind="ExternalInput")
    dr_w = {t: nc.dram_tensor(f"w{t}", [D, P], F32, kind="ExternalInput")
            for t in ("q", "k", "v")}
    dr_b = {t: nc.dram_tensor(f"b{t}", [P], F32, kind="ExternalInput")
            for t in ("q", "k", "v")}
    dr_out = nc.dram_tensor("out", [B_LOC, P, S], F32, kind="ExternalOutput")

    with tile.TileContext(nc) as tc:
        with (
            tc.tile_pool(name="const", bufs=1) as const,
            tc.tile_pool(name="stream", bufs=12) as stream,
            tc.tile_pool(name="proj", bufs=2) as proj,
            tc.tile_pool(name="ptp", bufs=14) as ptp,
            tc.tile_pool(name="stats", bufs=4) as stats,
            tc.tile_pool(name="recp", bufs=18) as recp,
            tc.tile_pool(name="osb", bufs=2) as osb,
            tc.tile_pool(name="ps_big", bufs=2, space="PSUM") as ps_big,
            tc.tile_pool(name="ps_acc", bufs=1, space="PSUM") as ps_acc,
        ):
            # ---- constants ----
            w_sb = {}
            b_sb = {}
            for t in ("q", "k", "v"):
                w_sb[t] = const.tile([P, DC, P], BF16, tag=f"w{t}", name=f"w{t}")
                # [D, P] -> [p, c, e]; cast f32->bf16 inline (SWDGE)
                nc.gpsimd.dma_start(
                    w_sb[t][:],
                    dr_w[t].ap().rearrange("(c p) e -> p c e", p=P))
                b_sb[t] = const.tile([P, 1], F32, tag=f"b{t}", name=f"b{t}")
                nc.sync.dma_start(
                    b_sb[t][:],
                    dr_b[t].ap().rearrange("(p o) -> p o", o=1))
            ident = const.tile([P, P], BF16, tag="ident", name="ident")
            make_identity(nc, ident[:])

            for b in range(B_LOC):
                # ---- Q projection (8 full-S chunks) ----
                halves = [ps_big.tile([P, 1024], F32, tag="big", name="q_ps")
                          for _ in range(2)]
                for c in range(DC):
                    x = stream.tile([P, S], BF16, tag="stream", name="x")
                    nc.gpsimd.dma_start(x[:], dr_in["q"].ap()[b, c * P:(c + 1) * P, :])
                    for h in range(2):
                        for s2 in range(2):
                            nc.tensor.matmul(
                                halves[h][:, s2 * 512:(s2 + 1) * 512],
                                lhsT=w_sb["q"][:, c, :],
                                rhs=x[:, h * 1024 + s2 * 512:
                                      h * 1024 + (s2 + 1) * 512],
                                start=(c == 0), stop=(c == DC - 1))
                qt = proj.tile([P, S], BF16, tag="qT", name="qT")
                for h in range(2):
                    nc.vector.tensor_scalar_add(
                        qt[:, h * 1024:(h + 1) * 1024],
                        halves[h][:], b_sb["q"][:])

                # ---- K projection (8 full-S chunks) ----
                khalves = [ps_big.tile([P, 1024], F32, tag="big", name="k_ps")
                           for _ in range(2)]
                for c in range(DC):
                    x = stream.tile([P, S], BF16, tag="stream", name="x")
                    nc.gpsimd.dma_start(x[:], dr_in["k"].ap()[b, c * P:(c + 1) * P, :])
                    for h in range(2):
                        for s2 in range(2):
                            nc.tensor.matmul(
                                khalves[h][:, s2 * 512:(s2 + 1) * 512],
                                lhsT=w_sb["k"][:, c, :],
                                rhs=x[:, h * 1024 + s2 * 512:
                                      h * 1024 + (s2 + 1) * 512],
                                start=(c == 0), stop=(c == DC - 1))
                kt = proj.tile([P, S], BF16, tag="kT", name="kT")
                for h in range(2):
                    nc.vector.tensor_scalar_add(
                        kt[:, h * 1024:(h + 1) * 1024],
                        khalves[h][:], b_sb["k"][:])

                # ---- scores + exp + Z for all k-chunks ----
                pts = []
                recs = []
                for kc in range(KC):
                    pt = ptp.tile([P, S], BF16, tag="pt", name="pt")
                    zz = stats.tile([P, 2], F32, tag="z", name="zz")
                    for h in range(2):
                        sc = ps_big.tile([P, 1024], F32, tag="big",
                                         name="sc_ps")
                        for s2 in range(2):
                            nc.tensor.matmul(
                                sc[:, s2 * 512:(s2 + 1) * 512],
                                lhsT=kt[:, kc * P:(kc + 1) * P],
                                rhs=qt[:, h * 1024 + s2 * 512:
                                       h * 1024 + (s2 + 1) * 512],
                                start=True, stop=True)
                        nc.scalar.activation(
                            pt[:, h * 1024:(h + 1) * 1024], sc[:],
                            func=mybir.ActivationFunctionType.Exp,
                            scale=SCALE, accum_out=zz[:, h:h + 1])
                    rec = recp.tile([P, 1], F32, tag="rec", name="rec")
                    nc.vector.tensor_reduce(
                        rec[:], zz[:], axis=mybir.AxisListType.X,
                        op=mybir.AluOpType.add)
                    nc.vector.reciprocal(rec[:], rec[:])
                    pts.append(pt)
                    recs.append(rec)

                # ---- V projection into the "acc" PSUM region (time-shared
                # VT -> transpose -> HT; never contends with "big"). The
                # v-chunk DMAs land while the scores/exp chain runs. ----
                vt_ps = ps_acc.tile([P, S], F32, tag="acc", name="vt_ps")
                for c in range(DC):
                    x = stream.tile([P, S], BF16, tag="stream", name="x")
                    nc.gpsimd.dma_start(
                        x[:], dr_in["v"].ap()[b, c * P:(c + 1) * P, :])
                    for r in range(4):
                        nc.tensor.matmul(
                            vt_ps[:, r * 512:(r + 1) * 512],
                            lhsT=w_sb["v"][:, c, :],
                            rhs=x[:, r * 512:(r + 1) * 512],
                            start=(c == 0), stop=(c == DC - 1))

                vt_sb = proj.tile([P, S], BF16, tag="vT", name="vT")
                for h in range(2):
                    nc.vector.tensor_scalar_add(
                        vt_sb[:, h * 1024:(h + 1) * 1024],
                        vt_ps[:, h * 1024:(h + 1) * 1024], b_sb["v"][:])
                v_sb = proj.tile([P, KC, P], BF16, tag="v", name="v")
                trt = ps_acc.tile([P, S], BF16, tag="acc", name="tr_ps")
                for idx in range(KC):
                    nc.tensor.transpose(
                        trt[:, idx * P:(idx + 1) * P],
                        vt_sb[:, idx * P:(idx + 1) * P], ident[:])
                for g in range(2):
                    nc.vector.tensor_copy(
                        v_sb[:, g * 8:(g + 1) * 8, :],
                        trt[:, g * 1024:(g + 1) * 1024].rearrange(
                            "p (a e) -> p a e", a=8))

                # ---- H^T accumulation over k-chunks ----
                ht = ps_acc.tile([P, S], F32, tag="acc", name="ht")
                for kc in range(KC):
                    vs = stats.tile([P, P], BF16, tag="vs", name="vs")
                    nc.vector.tensor_scalar_mul(
                        vs[:], v_sb[:, kc, :], recs[kc][:])
                    for st in range(4):
                        nc.tensor.matmul(
                            ht[:, st * 512:(st + 1) * 512],
                            lhsT=vs[:],
                            rhs=pts[kc][:, st * 512:(st + 1) * 512],
                            start=(kc == 0), stop=(kc == KC - 1))
                out_sb = osb.tile([P, S], F32, tag="osb", name="out_sb")
                nc.vector.tensor_copy(out_sb[:], ht[:])
                nc.sync.dma_start(dr_out.ap()[b], out_sb[:])

    nc.compile()
    return nc


def _get_nc():
    global _BUILT
    if _BUILT is None:
        _BUILT = build()
    return _BUILT


def kernel(inp_q, inp_k, inp_v, Wq_kernel, Wq_bias, Wk_kernel, Wk_bias,
           Wv_kernel, Wv_bias):
    from concourse.bass_utils import run_bass_kernel_spmd

    nc = _get_nc()

    inp = {"q": np.asarray(inp_q, dtype=np.float32),
           "k": np.asarray(inp_k, dtype=np.float32),
           "v": np.asarray(inp_v, dtype=np.float32)}
    w = {"q": np.ascontiguousarray(np.asarray(Wq_kernel, dtype=np.float32)),
         "k": np.ascontiguousarray(np.asarray(Wk_kernel, dtype=np.float32)),
         "v": np.ascontiguousarray(np.asarray(Wv_kernel, dtype=np.float32))}
    bias = {"q": np.ascontiguousarray(np.asarray(Wq_bias, dtype=np.float32)),
            "k": np.ascontiguousarray(np.asarray(Wk_bias, dtype=np.float32)),
            "v": np.ascontiguousarray(np.asarray(Wv_bias, dtype=np.float32))}

    in_maps = []
    for c in range(N_CORES):
        m = {}
        for t in ("q", "k", "v"):
            # [2, S, D] -> [2, D, S] contiguous (pure layout marshalling)
            m[f"{t}T"] = np.ascontiguousarray(
                inp[t][c * B_LOC:(c + 1) * B_LOC].transpose(0, 2, 1))
            m[f"w{t}"] = w[t]
            m[f"b{t}"] = bias[t]
        in_maps.append(m)

    res = run_bass_kernel_spmd(nc, in_maps, list(range(N_CORES)))

    out = np.empty((N_CORES * B_LOC, S, P), dtype=np.float32)
    for c in range(N_CORES):
        # [2, P, S] -> [2, S, P]
        out[c * B_LOC:(c + 1) * B_LOC] = (
            res.results[c]["out"].transpose(0, 2, 1))
    return out
